# revision 12
# baseline (speedup 1.0000x reference)
"""GAT (2-layer, mu/std heads) Trainium2 kernel — 8-core SPMD.

Sharding: nodes partitioned into 8 contiguous ranges (dst-sharding); edges
assigned to the core owning their dst, sorted by (dst-tile, src-half, src).
Per-layer halo exchange of bf16 node records via AllGather. Edge gather via
dma_gather (512B records by src from the global table). alpha_dst is fetched
on-chip: a transposed one-hot (dst-slot x edge) matmuls a per-tile alpha_dst
table, so no second DMA gather is needed. Scatter-add via one-hot matmul
with softmax denominators as extra matmul columns; W_mu/W_std projections
applied after aggregation.

Record layout (bf16, 256 elems = 512B):
  [0]=as0 [1]=as1 [2]=ad0 [3]=ad1 [4:68]=x_h0 [68]=1.0
  [132:196]=x_h1 [196]=1.0  (rest pad; layer2: h0/h1 are halves of h)
"""
import sys
sys.path.insert(0, '/opt/trn_rl_repo')
import numpy as np
import ml_dtypes

BF = ml_dtypes.bfloat16

# ---------------- problem constants (hardcoded per spec) ----------------
N = 50000
F_IN = 128
HID = 64
H = 2
Z = 32
NEG = 0.2
NCORES = 8
NPC = N // NCORES            # 6250 nodes per core
P = 128
NT = (NPC + P - 1) // P      # 49 dst tiles per core
NPCPAD = NT * P              # 6272
SENTROW = NPCPAD - 1         # per-core sentinel row (alpha = -1e30)
RECW = 256                   # record bf16 elems per node row (512 B)
HALFROWS = (NCORES // 2) * NPCPAD   # 25088 rows per half-table
NB = 32                      # blocks per gather batch
GMAXB = 8                    # max blocks (128 idx each) per dma_gather call
BIG = -1.0e30


# ---------------- host-side prep ----------------
def _prep_edges(edges):
    """Shard + sort by (tile, src-half, src) + pad; build packed index arrays.

    Returns (schedule [NT,2] int, half_flags, per-core dict)."""
    src = np.concatenate([edges[0].astype(np.int64), np.arange(N, dtype=np.int64)])
    dst = np.concatenate([edges[1].astype(np.int64), np.arange(N, dtype=np.int64)])
    core = dst // NPC
    dstl = dst - core * NPC
    tile = dstl >> 7
    src_row = (src // NPC) * NPCPAD + (src % NPC)   # padded global row
    half = (src_row >= HALFROWS).astype(np.int64)

    counts = np.zeros((NCORES, NT, 2), np.int64)
    np.add.at(counts, (core, tile, half), 1)
    blocks = (counts + P - 1) // P                   # [C, NT, 2]
    schedule = blocks.max(axis=0)                    # [NT, 2]
    schedule[:, 0] = np.maximum(schedule[:, 0], 1)   # >=1 block per tile
    nblk = int(schedule.sum())

    # flat block index of each (tile, half) group start
    grp_blocks = schedule.reshape(-1)                # [NT*2]
    grp_start = np.zeros(NT * 2, np.int64)
    grp_start[1:] = np.cumsum(grp_blocks)[:-1]
    grp_start = grp_start.reshape(NT, 2)

    half_flags = np.zeros(nblk, np.int64)
    for t in range(NT):
        half_flags[grp_start[t, 1]:grp_start[t, 1] + schedule[t, 1]] = 1

    per_core = []
    for c in range(NCORES):
        m = core == c
        key = tile[m] * 2 + half[m]
        srow_m = src_row[m]
        order = np.lexsort((srow_m, key))            # by group, then src asc
        key_s = key[order]
        srow_s = srow_m[order]
        dstl_s = dstl[m][order]
        half_s = half[m][order]
        cnt = counts[c].reshape(-1)                  # [NT*2]
        g0 = np.zeros(NT * 2, np.int64)
        g0[1:] = np.cumsum(cnt)[:-1]
        k = np.arange(key_s.size, dtype=np.int64) - g0[key_s]
        flat = (grp_start.reshape(-1)[key_s] + (k >> 7)) * P + (k & 127)

        # defaults: pad edges -> sentinel record of the matching half
        esrc = np.empty(nblk * P, np.int64)
        for t in range(NT):
            a0, a1 = grp_start[t, 0] * P, (grp_start[t, 0] + schedule[t, 0]) * P
            b0, b1 = grp_start[t, 1] * P, (grp_start[t, 1] + schedule[t, 1]) * P
            esrc[a0:a1] = SENTROW                    # core0 sentinel (half A)
            esrc[b0:b1] = (NCORES // 2) * NPCPAD + SENTROW - HALFROWS
        eslot = np.zeros(nblk * P, np.int64)
        esrc[flat] = srow_s - half_s * HALFROWS
        eslot[flat] = dstl_s & 127

        # gather-idx wrap-16 packing, replicated to 128 partitions
        def pack16(vals):
            n = vals.size
            t16 = np.zeros((16, n // 16), np.int16)
            t16[np.arange(n) % 16, np.arange(n) // 16] = vals.astype(np.int16)
            return np.ascontiguousarray(np.tile(t16, (8, 1)))

        per_core.append({
            "esrc16": pack16(esrc),
            "eslot": np.ascontiguousarray(
                eslot.reshape(nblk, P).T.astype(BF)),
            "eslotf": np.ascontiguousarray(eslot.astype(BF).reshape(1, -1)),
        })
    return schedule, half_flags, per_core


def _prep_weights(W1, a_src1, a_dst1, b1, W_mu, a_src_mu, a_dst_mu, b_mu,
                  W_std, a_src_std, a_dst_std, b_std):
    am1 = np.zeros((F_IN, 4), np.float32)
    am1[0:HID, 0] = a_src1[0]
    am1[HID:2 * HID, 1] = a_src1[1]
    am1[0:HID, 2] = a_dst1[0]
    am1[HID:2 * HID, 3] = a_dst1[1]
    am2 = np.zeros((2 * Z, 4), np.float32)
    am2[0:Z, 0] = a_src_mu[0]
    am2[Z:2 * Z, 1] = a_src_std[0]
    am2[0:Z, 2] = a_dst_mu[0]
    am2[Z:2 * Z, 3] = a_dst_std[0]
    # pack all node-local params into one [128, 648] f32 blob:
    # cols: w1t 0:128 | w1raw 128:256 | amask1 256:260 | wcat 260:388 (rows<64)
    #       amask2 388:392 (rows<64) | wmut 392:424 | wstdt 424:456
    #       b1rep 456:584 | bmurep 584:616 | bstdrep 616:648
    blob = np.zeros((P, 648), np.float32)
    blob[:, 0:128] = W1.T
    blob[:, 128:256] = W1
    blob[:, 256:260] = am1
    blob[0:2 * Z, 260:388] = np.vstack([W_mu, W_std])
    blob[0:2 * Z, 388:392] = am2
    blob[:, 392:424] = W_mu.T
    blob[:, 424:456] = W_std.T
    blob[:, 456:584] = np.tile(b1.astype(np.float32), (P, 1))
    blob[:, 584:616] = np.tile(b_mu.astype(np.float32), (P, 1))
    blob[:, 616:648] = np.tile(b_std.astype(np.float32), (P, 1))
    return {"wblob": np.ascontiguousarray(blob)}


# ---------------- device program ----------------
def _build_nc(schedule, half_flags):
    import concourse.bass as bass
    import concourse.mybir as mybir
    import concourse.tile as tile
    import concourse.bacc as bacc
    from concourse.masks import make_identity

    f32 = mybir.dt.float32
    bf16 = mybir.dt.bfloat16
    i16 = mybir.dt.int16
    AF = mybir.ActivationFunctionType
    OP = mybir.AluOpType
    nblk = int(schedule.sum())

    # flat block index -> dst tile
    tile_of_block = np.zeros(nblk, np.int64)
    b = 0
    for t in range(NT):
        for hf in range(2):
            for _ in range(int(schedule[t, hf])):
                tile_of_block[b] = t
                b += 1

    nc = bacc.Bacc("TRN2", target_bir_lowering=False, debug=False,
                   num_devices=NCORES)

    # ---- I/O ----
    xT = nc.dram_tensor("xT", [P, NPCPAD], f32, kind="ExternalInput")
    wblob_d = nc.dram_tensor("wblob", [P, 648], f32, kind="ExternalInput")
    esrc_d = nc.dram_tensor("esrc16", [P, nblk * 8], i16, kind="ExternalInput")
    eslot_d = nc.dram_tensor("eslot", [P, nblk], bf16, kind="ExternalInput")
    eslotf_d = nc.dram_tensor("eslotf", [1, nblk * P], bf16, kind="ExternalInput")
    musd_out = nc.dram_tensor("musd_out", [NPC, 2 * Z], f32, kind="ExternalOutput")

    with tile.TileContext(nc) as tc:
        with tc.tile_pool(name="dram", bufs=1, space="DRAM") as dram, \
             tc.tile_pool(name="const", bufs=1) as cp:
            rec1_slice = dram.tile([NPCPAD, RECW], bf16)
            rec1_full = dram.tile([NPCPAD * NCORES, RECW], bf16,
                                  addr_space="Shared")
            rec2_slice = dram.tile([NPCPAD, RECW], bf16)
            rec2_full = dram.tile([NPCPAD * NCORES, RECW], bf16,
                                  addr_space="Shared")

            # ---- constants ----
            iota_bf = cp.tile([P, P], bf16)
            nc.gpsimd.iota(iota_bf[:], pattern=[[1, P]], base=0,
                           channel_multiplier=0,
                           allow_small_or_imprecise_dtypes=True)
            iota_col = cp.tile([P, 1], f32)
            nc.gpsimd.iota(iota_col[:], pattern=[[1, 1]], base=0,
                           channel_multiplier=1,
                           allow_small_or_imprecise_dtypes=True)
            ident = cp.tile([P, P], f32)
            make_identity(nc, ident[:])
            sent_bf = cp.tile([1, 4], bf16)
            nc.gpsimd.memset(sent_bf[:], BIG)
            # per-tile alpha_dst tables (bf16), filled by the node phases
            adtab1 = cp.tile([P, NT * 2], bf16)
            adtab2 = cp.tile([P, NT * 2], bf16)

            wblob_s = cp.tile([P, 648], f32, name="wblob_s")
            nc.sync.dma_start(out=wblob_s[:], in_=wblob_d[:])
            w1t_s = wblob_s[:, 0:128]
            w1raw_s = wblob_s[:, 128:256]
            amask1_s = wblob_s[:, 256:260]
            wcat_s = wblob_s[0:2 * Z, 260:388]
            amask2_s = wblob_s[0:2 * Z, 388:392]
            wmut_s = wblob_s[:, 392:424]
            wstdt_s = wblob_s[:, 424:456]
            b1rep_s = wblob_s[:, 456:584]
            bmurep_s = wblob_s[:, 584:616]
            bstdrep_s = wblob_s[:, 616:648]

            # ---- u-vectors (alpha matvec weights), bf16 copies for L2 ----
            u1_s = cp.tile([F_IN, 4], f32)
            u2_s = cp.tile([F_IN, 4], bf16)
            with tc.tile_pool(name="ups", bufs=1, space="PSUM") as ups:
                u1_ps = ups.tile([F_IN, 4], f32)
                nc.tensor.matmul(out=u1_ps[:], lhsT=w1raw_s, rhs=amask1_s,
                                 start=True, stop=True)
                nc.vector.tensor_copy(out=u1_s[:], in_=u1_ps[:])
                u2_ps = ups.tile([F_IN, 4], f32)
                nc.tensor.matmul(out=u2_ps[:], lhsT=wcat_s,
                                 rhs=amask2_s, start=True, stop=True)
                nc.vector.tensor_copy(out=u2_s[:], in_=u2_ps[:])

            # ---- node phase 1 ----
            with tc.tile_pool(name="xtp", bufs=1) as xtp, \
                 tc.tile_pool(name="n1", bufs=3) as n1, \
                 tc.tile_pool(name="n1ps", bufs=2, space="PSUM") as n1ps:
                xT_s = xtp.tile([P, NPCPAD], f32)
                nc.sync.dma_start(out=xT_s[:], in_=xT[:])
                for T in range(NT):
                    lhs = xT_s[:, T * P:(T + 1) * P]
                    xp_ps = n1ps.tile([P, F_IN], f32)
                    a1_ps = n1ps.tile([P, 4], f32)
                    nc.tensor.matmul(out=xp_ps[:], lhsT=lhs, rhs=w1t_s,
                                     start=True, stop=True)
                    nc.tensor.matmul(out=a1_ps[:], lhsT=lhs, rhs=u1_s[:],
                                     start=True, stop=True)
                    rec_t = n1.tile([P, RECW], bf16)
                    nc.gpsimd.memset(rec_t[:], 1.0)
                    nc.vector.tensor_copy(
                        out=rec_t[:].rearrange("p (h q) -> p h q", q=P)[:, :, 4:4 + HID],
                        in_=xp_ps[:].rearrange("p (h c) -> p h c", c=HID))
                    nc.vector.tensor_copy(out=rec_t[:, 0:4], in_=a1_ps[:])
                    nc.vector.tensor_copy(out=adtab1[:, T * 2:T * 2 + 2],
                                          in_=a1_ps[:, 2:4])
                    nc.sync.dma_start(out=rec1_slice[T * P:(T + 1) * P, :],
                                      in_=rec_t[:])
                nc.sync.dma_start(out=rec1_slice[SENTROW:SENTROW + 1, 0:4],
                                  in_=sent_bf[:])

            # ---- AllGather 1 ----
            nc.gpsimd.collective_compute(
                "AllGather", OP.bypass,
                replica_groups=[list(range(NCORES))],
                ins=[rec1_slice[:]], outs=[rec1_full[:]])

            # ---- edge phase (shared for both layers) ----
            def edge_phase(layer, full_tab, adtab, normalize):
                # w width per block: L1 2*(65) ; L2 2*2*(65)
                ngrp = 2 if layer == 2 else 1
                ww = ngrp * 2 * (HID + 1)            # 130 / 260
                viewA = full_tab[0:HALFROWS, :]
                viewB = full_tab[HALFROWS:2 * HALFROWS, :]
                with tc.tile_pool(name=f"e{layer}", bufs=3) as ep, \
                     tc.tile_pool(name=f"e{layer}a", bufs=2) as epa, \
                     tc.tile_pool(name=f"n{layer}x", bufs=3) as np_, \
                     tc.tile_pool(name=f"e{layer}ps", bufs=2, space="PSUM") as eps, \
                     tc.tile_pool(name=f"ad{layer}ps", bufs=2, space="PSUM") as adps_p, \
                     tc.tile_pool(name=f"n{layer}xps", bufs=2, space="PSUM") as nps:
                    state = {"a0": None, "w": None, "b0": 0}

                    def emit_batch(b0):
                        bn = min(NB, nblk - b0)
                        esrc_t = ep.tile([P, NB * 8], i16, name=f"esrc{layer}")
                        dslot = ep.tile([P, NB], bf16, name=f"dslot{layer}")
                        eslotT = ep.tile([P, NB * P], bf16, name=f"eslT{layer}")
                        nc.sync.dma_start(out=esrc_t[:, 0:bn * 8],
                                          in_=esrc_d[:, b0 * 8:(b0 + bn) * 8])
                        nc.sync.dma_start(out=dslot[:, 0:bn],
                                          in_=eslot_d[:, b0:b0 + bn])
                        nc.sync.dma_start(
                            out=eslotT[:, 0:bn * P],
                            in_=eslotf_d[0:1, b0 * P:(b0 + bn) * P].to_broadcast(
                                [P, bn * P]))
                        rec_g = ep.tile([P, NB * RECW], bf16, name=f"rec_g{layer}")
                        # gather: per same-half run of blocks (alternate queues)
                        r0 = 0
                        while r0 < bn:
                            hf = half_flags[b0 + r0]
                            r1 = r0 + 1
                            while (r1 < bn and r1 - r0 < GMAXB
                                   and half_flags[b0 + r1] == hf):
                                r1 += 1
                            nrun = (r1 - r0) * P
                            nc.gpsimd.dma_gather(
                                out_ap=rec_g[:, r0 * RECW:r1 * RECW].rearrange(
                                    "p (g e) -> p g e", e=RECW),
                                in_ap=(viewB if hf else viewA),
                                idxs_ap=esrc_t[:, r0 * 8:r1 * 8],
                                num_idxs=nrun, num_idxs_reg=nrun,
                                elem_size=RECW)
                            r0 = r1
                        # transposed one-hot (dst-slot x edge): per-partition
                        # scalar compare -> DVE 4x fast path
                        a0T = epa.tile([P, NB * P], bf16, name=f"a0T{layer}")
                        nc.vector.tensor_scalar(
                            out=a0T[:, 0:bn * P],
                            in0=eslotT[:, 0:bn * P],
                            scalar1=iota_col[:, 0:1], scalar2=None,
                            op0=OP.is_equal)
                        ad_ps = adps_p.tile([P, NB * 2], f32, name=f"adps{layer}")
                        for o in range(bn):
                            t = int(tile_of_block[b0 + o])
                            nc.tensor.matmul(
                                out=ad_ps[:, o * 2:(o + 1) * 2],
                                lhsT=a0T[:, o * P:(o + 1) * P],
                                rhs=adtab[:, t * 2:t * 2 + 2],
                                start=True, stop=True)
                        # t = as + ad ; u = max(.2t, t) ; p = exp(u)
                        tt = ep.tile([P, NB * 2], bf16, name=f"tt{layer}")
                        nc.vector.tensor_tensor(
                            out=tt[:, 0:bn * 2].rearrange("p (b h) -> p b h", h=2),
                            in0=rec_g[:, 0:bn * RECW].rearrange(
                                "p (b r) -> p b r", r=RECW)[:, :, 0:2],
                            in1=ad_ps[:, 0:bn * 2].rearrange(
                                "p (b h) -> p b h", h=2),
                            op=OP.add)
                        uu = ep.tile([P, NB * 2], bf16, name=f"uu{layer}")
                        nc.vector.scalar_tensor_tensor(
                            out=uu[:, 0:bn * 2], in0=tt[:, 0:bn * 2],
                            scalar=NEG, in1=tt[:, 0:bn * 2],
                            op0=OP.mult, op1=OP.max)
                        pp = ep.tile([P, NB * 2], bf16, name=f"pp{layer}")
                        nc.scalar.activation(pp[:, 0:bn * 2], uu[:, 0:bn * 2],
                                             AF.Exp)
                        # A0 one-hot
                        a0 = epa.tile([P, NB * P], bf16, name=f"a0_{layer}")
                        nc.vector.tensor_tensor(
                            out=a0[:, 0:bn * P].rearrange("p (b r) -> p b r", r=P),
                            in0=dslot[:, 0:bn][:, :, None].to_broadcast([P, bn, P]),
                            in1=iota_bf[:][:, None, :].to_broadcast([P, bn, P]),
                            op=OP.is_equal)
                        # w build
                        w = epa.tile([P, NB * ww], bf16, name=f"w{layer}")
                        rec3 = rec_g[:, 0:bn * RECW].rearrange(
                            "p (b r) -> p b r", r=RECW)
                        rec4 = rec3.rearrange("p b (h q) -> p b h q", q=P)[
                            :, :, :, 4:4 + HID + 1]
                        if layer == 1:
                            in1 = pp[:, 0:bn * 2].rearrange(
                                "p (b h) -> p b h", h=2)[:, :, :, None].to_broadcast(
                                [P, bn, 2, HID + 1])
                            wv = w[:, 0:bn * ww].rearrange(
                                "p (b h c) -> p b h c", h=2, c=HID + 1)
                            nc.vector.tensor_tensor(out=wv, in0=rec4, in1=in1,
                                                    op=OP.mult)
                        else:
                            pp3 = pp[:, 0:bn * 2].rearrange(
                                "p (b g) -> p b g", g=2)
                            wv4 = w[:, 0:bn * ww].rearrange(
                                "p (b g hc) -> p b g hc", g=2, hc=2 * (HID + 1))
                            for g, eng in ((0, nc.vector), (1, nc.gpsimd)):
                                eng.tensor_tensor(
                                    out=wv4[:, :, g].rearrange(
                                        "p b (h c) -> p b h c", c=HID + 1),
                                    in0=rec4,
                                    in1=pp3[:, :, g:g + 1][:, :, :, None].to_broadcast(
                                        [P, bn, 2, HID + 1]),
                                    op=OP.mult)
                        state["a0"], state["w"], state["b0"] = a0, w, b0

                    B = 0
                    for T in range(NT):
                        ps = eps.tile([P, ww], f32, name=f"acc{layer}")
                        kb = int(schedule[T].sum())
                        for j in range(kb):
                            if state["a0"] is None or B >= state["b0"] + NB:
                                emit_batch(B)
                            o = B - state["b0"]
                            nc.tensor.matmul(
                                out=ps[:],
                                lhsT=state["a0"][:, o * P:(o + 1) * P],
                                rhs=state["w"][:, o * ww:(o + 1) * ww],
                                start=(j == 0), stop=(j == kb - 1))
                            B += 1
                        normalize(ps, T, np_, nps)

            # ---- normalize callbacks ----
            def norm1(ps, T, np_, nps):
                ps3 = ps[:].rearrange("p (h c) -> p h c", c=HID + 1)
                se = np_.tile([P, 2], f32, name="se1")
                nc.vector.tensor_scalar_add(
                    se[:].rearrange("p (h o) -> p h o", o=1),
                    ps3[:, :, HID:HID + 1], 1e-30)
                rs = np_.tile([P, 2], f32, name="rs1")
                nc.vector.reciprocal(rs[:], se[:])
                h_f = np_.tile([P, F_IN], f32, name="h_f")
                hv = h_f[:].rearrange("p (h c) -> p h c", c=HID)
                nc.vector.tensor_tensor(
                    out=hv, in0=ps3[:, :, 0:HID],
                    in1=rs[:].rearrange("p (h o) -> p h o", o=1).to_broadcast(
                        [P, 2, HID]),
                    op=OP.mult)
                nc.vector.tensor_tensor(out=h_f[:], in0=h_f[:], in1=b1rep_s,
                                        op=OP.add)
                rec2_t = np_.tile([P, RECW], bf16, name="rec2t")
                nc.gpsimd.memset(rec2_t[:], 1.0)
                nc.scalar.activation(
                    rec2_t[:].rearrange("p (h q) -> p h q", q=P)[:, :, 4:4 + HID],
                    h_f[:].rearrange("p (h c) -> p h c", c=HID), AF.Relu)
                # relu'd h also needed in f32 for the transpose/alpha matvec
                hr_f = np_.tile([P, F_IN], f32, name="hr_f")
                nc.scalar.activation(hr_f[:], h_f[:], AF.Relu)
                hT_ps = nps.tile([P, P], f32, name="hTps")
                nc.tensor.transpose(out=hT_ps[:], in_=hr_f[:], identity=ident[:])
                hT_s = np_.tile([P, P], bf16, name="hTs")
                nc.vector.tensor_copy(out=hT_s[:], in_=hT_ps[:])
                a2_ps = nps.tile([P, 4], f32, name="a2ps")
                nc.tensor.matmul(out=a2_ps[:], lhsT=hT_s[:], rhs=u2_s[:],
                                 start=True, stop=True)
                nc.vector.tensor_copy(out=rec2_t[:, 0:4], in_=a2_ps[:])
                nc.vector.tensor_copy(out=adtab2[:, T * 2:T * 2 + 2],
                                      in_=a2_ps[:, 2:4])
                nc.sync.dma_start(out=rec2_slice[T * P:(T + 1) * P, :],
                                  in_=rec2_t[:])

            def norm2(ps, T, np_, nps):
                ps3 = ps[:].rearrange("p (g c) -> p g c", c=2 * (HID + 1))
                se = np_.tile([P, 2], f32, name="se2")
                nc.vector.tensor_scalar_add(
                    se[:].rearrange("p (g o) -> p g o", o=1),
                    ps3[:, :, HID:HID + 1], 1e-30)
                rs = np_.tile([P, 2], f32, name="rs2")
                nc.vector.reciprocal(rs[:], se[:])
                agg = np_.tile([P, 2 * F_IN], f32, name="agg")
                nc.vector.tensor_tensor(
                    out=agg[:].rearrange("p (g h c) -> p g h c", g=2, c=HID),
                    in0=ps3[:].rearrange("p g (h c) -> p g h c", c=HID + 1)[
                        :, :, :, 0:HID],
                    in1=rs[:].rearrange("p (g o) -> p g o", o=1)[
                        :, :, :, None].to_broadcast([P, 2, 2, HID]),
                    op=OP.mult)
                rows = min(P, NPC - T * P)
                for gi, (wt_s, brep_s) in enumerate(
                        ((wmut_s, bmurep_s), (wstdt_s, bstdrep_s))):
                    aT_ps = nps.tile([P, P], f32, name="aTps")
                    nc.tensor.transpose(out=aT_ps[:],
                                        in_=agg[:, gi * F_IN:(gi + 1) * F_IN],
                                        identity=ident[:])
                    aT_s = np_.tile([P, P], f32, name="aTs")
                    nc.vector.tensor_copy(out=aT_s[:], in_=aT_ps[:])
                    pr_ps = nps.tile([P, Z], f32, name="prps")
                    nc.tensor.matmul(out=pr_ps[:], lhsT=aT_s[:], rhs=wt_s[:],
                                     start=True, stop=True)
                    o_s = np_.tile([P, Z], f32, name="outs")
                    nc.vector.tensor_tensor(out=o_s[:], in0=pr_ps[:],
                                            in1=brep_s[:], op=OP.add)
                    nc.sync.dma_start(
                        out=musd_out[T * P:T * P + rows,
                                     gi * Z:(gi + 1) * Z],
                        in_=o_s[0:rows, :])

            edge_phase(1, rec1_full, adtab1, norm1)

            # sentinel for layer-2 table (after all norm1 writes)
            nc.sync.dma_start(out=rec2_slice[SENTROW:SENTROW + 1, 0:4],
                              in_=sent_bf[:])

            # ---- AllGather 2 ----
            nc.gpsimd.collective_compute(
                "AllGather", OP.bypass,
                replica_groups=[list(range(NCORES))],
                ins=[rec2_slice[:]], outs=[rec2_full[:]])

            edge_phase(2, rec2_full, adtab2, norm2)

    nc.compile()
    return nc


# ---------------- runner ----------------
_CACHE = {}


def _get_runner(schedule, half_flags):
    key = tuple(schedule.reshape(-1).tolist())
    if key not in _CACHE:
        nc = _build_nc(schedule, half_flags)
        _CACHE[key] = (nc, {})
    return _CACHE[key]


def run_on_hw(inputs_per_core, schedule, half_flags):
    import jax
    from concourse import bass2jax
    nc, captured = _get_runner(schedule, half_flags)
    orig_jit = jax.jit

    def cap_jit(fun, **kw):
        j = orig_jit(fun, **kw)
        captured['fn'] = j
        return j
    jax.jit = cap_jit
    try:
        results = bass2jax.run_bass_via_pjrt(nc, inputs_per_core, n_cores=NCORES)
    finally:
        jax.jit = orig_jit
    return results, captured.get('fn'), nc


def make_inputs_per_core(features, edges, wp):
    schedule, half_flags, per_core = _prep_edges(np.asarray(edges))
    feats = np.asarray(features, np.float32)
    ins = []
    for c in range(NCORES):
        xTs = np.zeros((P, NPCPAD), np.float32)
        xTs[:, 0:NPC] = feats[c * NPC:(c + 1) * NPC].T
        ins.append({"xT": xTs, **wp, **per_core[c]})
    return schedule, half_flags, ins


def kernel(features, edges, W1, a_src1, a_dst1, b1, W_mu, a_src_mu, a_dst_mu,
           b_mu, W_std, a_src_std, a_dst_std, b_std):
    wp = _prep_weights(np.asarray(W1), np.asarray(a_src1), np.asarray(a_dst1),
                       np.asarray(b1), np.asarray(W_mu), np.asarray(a_src_mu),
                       np.asarray(a_dst_mu), np.asarray(b_mu), np.asarray(W_std),
                       np.asarray(a_src_std), np.asarray(a_dst_std),
                       np.asarray(b_std))
    schedule, half_flags, ins = make_inputs_per_core(features, edges, wp)
    results, _, _ = run_on_hw(ins, schedule, half_flags)
    musd = np.concatenate([results[c]["musd_out"] for c in range(NCORES)],
                          axis=0)
    return (np.ascontiguousarray(musd[:, 0:Z]),
            np.ascontiguousarray(musd[:, Z:2 * Z]))


# revision 13
# speedup vs baseline: 1.3879x; 1.3879x over previous
"""GAT (2-layer, mu/std heads) Trainium2 kernel — 8-core SPMD.

Sharding: nodes partitioned into 8 contiguous ranges (dst-sharding); edges
assigned to the core owning their dst, sorted by (dst-tile, src-half, src).
Per-layer halo exchange of bf16 node records via AllGather. Edge gather via
dma_gather (512B records by src from the global table). alpha_dst is fetched
on-chip: a transposed one-hot (dst-slot x edge) matmuls a per-tile alpha_dst
table, so no second DMA gather is needed. Scatter-add via one-hot matmul
with softmax denominators as extra matmul columns; W_mu/W_std projections
applied after aggregation.

Record layout (bf16, 256 elems = 512B):
  [0]=as0 [1]=as1 [2]=ad0 [3]=ad1 [4:68]=x_h0 [68]=1.0
  [132:196]=x_h1 [196]=1.0  (rest pad; layer2: h0/h1 are halves of h)
"""
import sys
sys.path.insert(0, '/opt/trn_rl_repo')
import numpy as np
import ml_dtypes

BF = ml_dtypes.bfloat16

# ---------------- problem constants (hardcoded per spec) ----------------
N = 50000
F_IN = 128
HID = 64
H = 2
Z = 32
NEG = 0.2
NCORES = 8
NPC = N // NCORES            # 6250 nodes per core
P = 128
NT = (NPC + P - 1) // P      # 49 dst tiles per core
NPCPAD = NT * P              # 6272
SENTROW = NPCPAD - 1         # per-core sentinel row (alpha = -1e30)
RECW = 256                   # record bf16 elems per node row (512 B)
HALFROWS = (NCORES // 2) * NPCPAD   # 25088 rows per half-table
NB = 32                      # blocks per gather batch
GMAXB = 8                    # max blocks (128 idx each) per dma_gather call
BIG = -1.0e30


# ---------------- host-side prep ----------------
def _prep_edges(edges):
    """Shard + sort by (tile, src-half, src) + pad; build packed index arrays.

    Returns (schedule [NT,2] int, half_flags, per-core dict)."""
    src = np.concatenate([edges[0].astype(np.int64), np.arange(N, dtype=np.int64)])
    dst = np.concatenate([edges[1].astype(np.int64), np.arange(N, dtype=np.int64)])
    core = dst // NPC
    dstl = dst - core * NPC
    tile = dstl >> 7
    src_row = (src // NPC) * NPCPAD + (src % NPC)   # padded global row
    half = (src_row >= HALFROWS).astype(np.int64)

    counts = np.zeros((NCORES, NT, 2), np.int64)
    np.add.at(counts, (core, tile, half), 1)
    blocks = (counts + P - 1) // P                   # [C, NT, 2]
    schedule = blocks.max(axis=0)                    # [NT, 2]
    schedule[:, 0] = np.maximum(schedule[:, 0], 1)   # >=1 block per tile
    nblk = int(schedule.sum())

    # flat block index of each (tile, half) group start
    grp_blocks = schedule.reshape(-1)                # [NT*2]
    grp_start = np.zeros(NT * 2, np.int64)
    grp_start[1:] = np.cumsum(grp_blocks)[:-1]
    grp_start = grp_start.reshape(NT, 2)

    half_flags = np.zeros(nblk, np.int64)
    for t in range(NT):
        half_flags[grp_start[t, 1]:grp_start[t, 1] + schedule[t, 1]] = 1

    per_core = []
    for c in range(NCORES):
        m = core == c
        key = tile[m] * 2 + half[m]
        srow_m = src_row[m]
        order = np.lexsort((srow_m, key))            # by group, then src asc
        key_s = key[order]
        srow_s = srow_m[order]
        dstl_s = dstl[m][order]
        half_s = half[m][order]
        cnt = counts[c].reshape(-1)                  # [NT*2]
        g0 = np.zeros(NT * 2, np.int64)
        g0[1:] = np.cumsum(cnt)[:-1]
        k = np.arange(key_s.size, dtype=np.int64) - g0[key_s]
        flat = (grp_start.reshape(-1)[key_s] + (k >> 7)) * P + (k & 127)

        # defaults: pad edges -> sentinel record of the matching half
        esrc = np.empty(nblk * P, np.int64)
        for t in range(NT):
            a0, a1 = grp_start[t, 0] * P, (grp_start[t, 0] + schedule[t, 0]) * P
            b0, b1 = grp_start[t, 1] * P, (grp_start[t, 1] + schedule[t, 1]) * P
            esrc[a0:a1] = SENTROW                    # core0 sentinel (half A)
            esrc[b0:b1] = (NCORES // 2) * NPCPAD + SENTROW - HALFROWS
        eslot = np.zeros(nblk * P, np.int64)
        esrc[flat] = srow_s - half_s * HALFROWS
        eslot[flat] = dstl_s & 127

        # gather-idx wrap-16 packing, replicated to 128 partitions
        def pack16(vals):
            n = vals.size
            t16 = np.zeros((16, n // 16), np.int16)
            t16[np.arange(n) % 16, np.arange(n) // 16] = vals.astype(np.int16)
            return np.ascontiguousarray(np.tile(t16, (8, 1)))

        per_core.append({
            "esrc16": pack16(esrc),
            "eslot": np.ascontiguousarray(
                eslot.reshape(nblk, P).T.astype(BF)),
            "eslotf": np.ascontiguousarray(eslot.astype(BF).reshape(1, -1)),
        })
    return schedule, half_flags, per_core


def _prep_weights(W1, a_src1, a_dst1, b1, W_mu, a_src_mu, a_dst_mu, b_mu,
                  W_std, a_src_std, a_dst_std, b_std):
    am1 = np.zeros((F_IN, 4), np.float32)
    am1[0:HID, 0] = a_src1[0]
    am1[HID:2 * HID, 1] = a_src1[1]
    am1[0:HID, 2] = a_dst1[0]
    am1[HID:2 * HID, 3] = a_dst1[1]
    am2 = np.zeros((2 * Z, 4), np.float32)
    am2[0:Z, 0] = a_src_mu[0]
    am2[Z:2 * Z, 1] = a_src_std[0]
    am2[0:Z, 2] = a_dst_mu[0]
    am2[Z:2 * Z, 3] = a_dst_std[0]
    # pack all node-local params into one [128, 648] f32 blob:
    # cols: w1t 0:128 | w1raw 128:256 | amask1 256:260 | wcat 260:388 (rows<64)
    #       amask2 388:392 (rows<64) | wmut 392:424 | wstdt 424:456
    #       b1rep 456:584 | bmurep 584:616 | bstdrep 616:648
    blob = np.zeros((P, 648), np.float32)
    blob[:, 0:128] = W1.T
    blob[:, 128:256] = W1
    blob[:, 256:260] = am1
    blob[0:2 * Z, 260:388] = np.vstack([W_mu, W_std])
    blob[0:2 * Z, 388:392] = am2
    blob[:, 392:424] = W_mu.T
    blob[:, 424:456] = W_std.T
    blob[:, 456:584] = np.tile(b1.astype(np.float32), (P, 1))
    blob[:, 584:616] = np.tile(b_mu.astype(np.float32), (P, 1))
    blob[:, 616:648] = np.tile(b_std.astype(np.float32), (P, 1))
    return {"wblob": np.ascontiguousarray(blob)}


# ---------------- device program ----------------
def _build_nc(schedule, half_flags):
    import concourse.bass as bass
    import concourse.mybir as mybir
    import concourse.tile as tile
    import concourse.bacc as bacc
    from concourse.masks import make_identity

    f32 = mybir.dt.float32
    bf16 = mybir.dt.bfloat16
    i16 = mybir.dt.int16
    AF = mybir.ActivationFunctionType
    OP = mybir.AluOpType
    nblk = int(schedule.sum())

    # flat block index -> dst tile
    tile_of_block = np.zeros(nblk, np.int64)
    b = 0
    for t in range(NT):
        for hf in range(2):
            for _ in range(int(schedule[t, hf])):
                tile_of_block[b] = t
                b += 1

    nc = bacc.Bacc("TRN2", target_bir_lowering=False, debug=False,
                   num_devices=NCORES)

    # ---- I/O ----
    xT = nc.dram_tensor("xT", [P, NPCPAD], f32, kind="ExternalInput")
    wblob_d = nc.dram_tensor("wblob", [P, 648], f32, kind="ExternalInput")
    esrc_d = nc.dram_tensor("esrc16", [P, nblk * 8], i16, kind="ExternalInput")
    eslot_d = nc.dram_tensor("eslot", [P, nblk], bf16, kind="ExternalInput")
    eslotf_d = nc.dram_tensor("eslotf", [1, nblk * P], bf16, kind="ExternalInput")
    musd_out = nc.dram_tensor("musd_out", [NPC, 2 * Z], f32, kind="ExternalOutput")

    with tile.TileContext(nc) as tc:
        with tc.tile_pool(name="dram", bufs=1, space="DRAM") as dram, \
             tc.tile_pool(name="const", bufs=1) as cp:
            rec1_slice = dram.tile([NPCPAD, RECW], bf16)
            rec1_full = dram.tile([NPCPAD * NCORES, RECW], bf16,
                                  addr_space="Shared")
            rec2_slice = dram.tile([NPCPAD, RECW], bf16)
            rec2_full = dram.tile([NPCPAD * NCORES, RECW], bf16,
                                  addr_space="Shared")

            # ---- constants ----
            iota_bf = cp.tile([P, P], bf16)
            nc.gpsimd.iota(iota_bf[:], pattern=[[1, P]], base=0,
                           channel_multiplier=0,
                           allow_small_or_imprecise_dtypes=True)
            iota_col = cp.tile([P, 1], f32)
            nc.gpsimd.iota(iota_col[:], pattern=[[1, 1]], base=0,
                           channel_multiplier=1,
                           allow_small_or_imprecise_dtypes=True)
            ident = cp.tile([P, P], f32)
            make_identity(nc, ident[:])
            sent_bf = cp.tile([1, 4], bf16)
            nc.gpsimd.memset(sent_bf[:], BIG)
            # per-tile alpha_dst tables (bf16), filled by the node phases
            adtab1 = cp.tile([P, NT * 2], bf16)
            adtab2 = cp.tile([P, NT * 2], bf16)

            wblob_s = cp.tile([P, 648], f32, name="wblob_s")
            nc.sync.dma_start(out=wblob_s[:], in_=wblob_d[:])
            w1t_s = wblob_s[:, 0:128]
            w1raw_s = wblob_s[:, 128:256]
            amask1_s = wblob_s[:, 256:260]
            wcat_s = wblob_s[0:2 * Z, 260:388]
            amask2_s = wblob_s[0:2 * Z, 388:392]
            wmut_s = wblob_s[:, 392:424]
            wstdt_s = wblob_s[:, 424:456]
            b1rep_s = wblob_s[:, 456:584]
            bmurep_s = wblob_s[:, 584:616]
            bstdrep_s = wblob_s[:, 616:648]

            # ---- u-vectors (alpha matvec weights), bf16 copies for L2 ----
            u1_s = cp.tile([F_IN, 4], f32)
            u2_s = cp.tile([F_IN, 4], bf16)
            with tc.tile_pool(name="ups", bufs=1, space="PSUM") as ups:
                u1_ps = ups.tile([F_IN, 4], f32)
                nc.tensor.matmul(out=u1_ps[:], lhsT=w1raw_s, rhs=amask1_s,
                                 start=True, stop=True)
                nc.vector.tensor_copy(out=u1_s[:], in_=u1_ps[:])
                u2_ps = ups.tile([F_IN, 4], f32)
                nc.tensor.matmul(out=u2_ps[:], lhsT=wcat_s,
                                 rhs=amask2_s, start=True, stop=True)
                nc.vector.tensor_copy(out=u2_s[:], in_=u2_ps[:])

            # ---- node phase 1 ----
            with tc.tile_pool(name="xtp", bufs=1) as xtp, \
                 tc.tile_pool(name="n1", bufs=3) as n1, \
                 tc.tile_pool(name="n1ps", bufs=2, space="PSUM") as n1ps:
                xT_s = xtp.tile([P, NPCPAD], f32)
                nc.sync.dma_start(out=xT_s[:], in_=xT[:])
                for T in range(NT):
                    lhs = xT_s[:, T * P:(T + 1) * P]
                    xp_ps = n1ps.tile([P, F_IN], f32)
                    a1_ps = n1ps.tile([P, 4], f32)
                    nc.tensor.matmul(out=xp_ps[:], lhsT=lhs, rhs=w1t_s,
                                     start=True, stop=True)
                    nc.tensor.matmul(out=a1_ps[:], lhsT=lhs, rhs=u1_s[:],
                                     start=True, stop=True)
                    rec_t = n1.tile([P, RECW], bf16)
                    nc.gpsimd.memset(rec_t[:], 1.0)
                    nc.vector.tensor_copy(
                        out=rec_t[:].rearrange("p (h q) -> p h q", q=P)[:, :, 4:4 + HID],
                        in_=xp_ps[:].rearrange("p (h c) -> p h c", c=HID))
                    nc.vector.tensor_copy(out=rec_t[:, 0:4], in_=a1_ps[:])
                    nc.vector.tensor_copy(out=adtab1[:, T * 2:T * 2 + 2],
                                          in_=a1_ps[:, 2:4])
                    nc.sync.dma_start(out=rec1_slice[T * P:(T + 1) * P, :],
                                      in_=rec_t[:])
                nc.sync.dma_start(out=rec1_slice[SENTROW:SENTROW + 1, 0:4],
                                  in_=sent_bf[:])

            # ---- AllGather 1 ----
            nc.gpsimd.collective_compute(
                "AllGather", OP.bypass,
                replica_groups=[list(range(NCORES))],
                ins=[rec1_slice[:]], outs=[rec1_full[:]])

            # ---- edge phase (shared for both layers) ----
            def edge_phase(layer, full_tab, adtab, normalize):
                # w width per block: L1 2*(65) ; L2 2*2*(65)
                ngrp = 2 if layer == 2 else 1
                ww = ngrp * 2 * (HID + 1)            # 130 / 260
                viewA = full_tab[0:HALFROWS, :]
                viewB = full_tab[HALFROWS:2 * HALFROWS, :]
                with tc.tile_pool(name=f"e{layer}", bufs=3) as ep, \
                     tc.tile_pool(name=f"e{layer}a", bufs=2) as epa, \
                     tc.tile_pool(name=f"n{layer}x", bufs=3) as np_, \
                     tc.tile_pool(name=f"e{layer}ps", bufs=2, space="PSUM") as eps, \
                     tc.tile_pool(name=f"ad{layer}ps", bufs=2, space="PSUM") as adps_p, \
                     tc.tile_pool(name=f"n{layer}xps", bufs=2, space="PSUM") as nps:
                    state = {"a0": None, "w": None, "b0": 0}

                    def emit_batch(b0):
                        bn = min(NB, nblk - b0)
                        esrc_t = ep.tile([P, NB * 8], i16, name=f"esrc{layer}")
                        dslot = ep.tile([P, NB], bf16, name=f"dslot{layer}")
                        eslotT = ep.tile([P, NB * P], bf16, name=f"eslT{layer}")
                        nc.sync.dma_start(out=esrc_t[:, 0:bn * 8],
                                          in_=esrc_d[:, b0 * 8:(b0 + bn) * 8])
                        nc.sync.dma_start(out=dslot[:, 0:bn],
                                          in_=eslot_d[:, b0:b0 + bn])
                        nc.sync.dma_start(
                            out=eslotT[:, 0:bn * P],
                            in_=eslotf_d[0:1, b0 * P:(b0 + bn) * P].to_broadcast(
                                [P, bn * P]))
                        rec_g = ep.tile([P, NB * RECW], bf16, name=f"rec_g{layer}")
                        # gather: per same-half run of blocks (alternate queues)
                        r0 = 0
                        while r0 < bn:
                            hf = half_flags[b0 + r0]
                            r1 = r0 + 1
                            while (r1 < bn and r1 - r0 < GMAXB
                                   and half_flags[b0 + r1] == hf):
                                r1 += 1
                            nrun = (r1 - r0) * P
                            nc.gpsimd.dma_gather(
                                out_ap=rec_g[:, r0 * RECW:r1 * RECW].rearrange(
                                    "p (g e) -> p g e", e=RECW),
                                in_ap=(viewB if hf else viewA),
                                idxs_ap=esrc_t[:, r0 * 8:r1 * 8],
                                num_idxs=nrun, num_idxs_reg=nrun,
                                elem_size=RECW)
                            r0 = r1
                        # transposed one-hot (dst-slot x edge): per-partition
                        # scalar compare -> DVE 4x fast path
                        a0T = epa.tile([P, NB * P], bf16, name=f"a0T{layer}")
                        nc.vector.tensor_scalar(
                            out=a0T[:, 0:bn * P],
                            in0=eslotT[:, 0:bn * P],
                            scalar1=iota_col[:, 0:1], scalar2=None,
                            op0=OP.is_equal)
                        ad_ps = adps_p.tile([P, NB * 2], f32, name=f"adps{layer}")
                        for o in range(bn):
                            t = int(tile_of_block[b0 + o])
                            nc.tensor.matmul(
                                out=ad_ps[:, o * 2:(o + 1) * 2],
                                lhsT=a0T[:, o * P:(o + 1) * P],
                                rhs=adtab[:, t * 2:t * 2 + 2],
                                start=True, stop=True)
                        # t = as + ad ; u = max(.2t, t) ; p = exp(u)
                        tt = ep.tile([P, NB * 2], bf16, name=f"tt{layer}")
                        nc.vector.tensor_tensor(
                            out=tt[:, 0:bn * 2].rearrange("p (b h) -> p b h", h=2),
                            in0=rec_g[:, 0:bn * RECW].rearrange(
                                "p (b r) -> p b r", r=RECW)[:, :, 0:2],
                            in1=ad_ps[:, 0:bn * 2].rearrange(
                                "p (b h) -> p b h", h=2),
                            op=OP.add)
                        uu = ep.tile([P, NB * 2], bf16, name=f"uu{layer}")
                        nc.vector.scalar_tensor_tensor(
                            out=uu[:, 0:bn * 2], in0=tt[:, 0:bn * 2],
                            scalar=NEG, in1=tt[:, 0:bn * 2],
                            op0=OP.mult, op1=OP.max)
                        pp = ep.tile([P, NB * 2], bf16, name=f"pp{layer}")
                        nc.scalar.activation(pp[:, 0:bn * 2], uu[:, 0:bn * 2],
                                             AF.Exp)
                        # A0 one-hot
                        a0 = epa.tile([P, NB * P], bf16, name=f"a0_{layer}")
                        nc.vector.tensor_tensor(
                            out=a0[:, 0:bn * P].rearrange("p (b r) -> p b r", r=P),
                            in0=dslot[:, 0:bn][:, :, None].to_broadcast([P, bn, P]),
                            in1=iota_bf[:][:, None, :].to_broadcast([P, bn, P]),
                            op=OP.is_equal)
                        # w build
                        w = epa.tile([P, NB * ww], bf16, name=f"w{layer}")
                        rec3 = rec_g[:, 0:bn * RECW].rearrange(
                            "p (b r) -> p b r", r=RECW)
                        rec4 = rec3.rearrange("p b (h q) -> p b h q", q=P)[
                            :, :, :, 4:4 + HID + 1]
                        if layer == 1:
                            in1 = pp[:, 0:bn * 2].rearrange(
                                "p (b h) -> p b h", h=2)[:, :, :, None].to_broadcast(
                                [P, bn, 2, HID + 1])
                            wv = w[:, 0:bn * ww].rearrange(
                                "p (b h c) -> p b h c", h=2, c=HID + 1)
                            nc.vector.tensor_tensor(out=wv, in0=rec4, in1=in1,
                                                    op=OP.mult)
                        else:
                            pp3 = pp[:, 0:bn * 2].rearrange(
                                "p (b g) -> p b g", g=2)
                            wv4 = w[:, 0:bn * ww].rearrange(
                                "p (b g hc) -> p b g hc", g=2, hc=2 * (HID + 1))
                            for g, eng in ((0, nc.vector), (1, nc.vector)):
                                eng.tensor_tensor(
                                    out=wv4[:, :, g].rearrange(
                                        "p b (h c) -> p b h c", c=HID + 1),
                                    in0=rec4,
                                    in1=pp3[:, :, g:g + 1][:, :, :, None].to_broadcast(
                                        [P, bn, 2, HID + 1]),
                                    op=OP.mult)
                        state["a0"], state["w"], state["b0"] = a0, w, b0

                    B = 0
                    for T in range(NT):
                        ps = eps.tile([P, ww], f32, name=f"acc{layer}")
                        kb = int(schedule[T].sum())
                        for j in range(kb):
                            if state["a0"] is None or B >= state["b0"] + NB:
                                emit_batch(B)
                            o = B - state["b0"]
                            nc.tensor.matmul(
                                out=ps[:],
                                lhsT=state["a0"][:, o * P:(o + 1) * P],
                                rhs=state["w"][:, o * ww:(o + 1) * ww],
                                start=(j == 0), stop=(j == kb - 1))
                            B += 1
                        normalize(ps, T, np_, nps)

            # ---- normalize callbacks ----
            def norm1(ps, T, np_, nps):
                ps3 = ps[:].rearrange("p (h c) -> p h c", c=HID + 1)
                se = np_.tile([P, 2], f32, name="se1")
                nc.vector.tensor_scalar_add(
                    se[:].rearrange("p (h o) -> p h o", o=1),
                    ps3[:, :, HID:HID + 1], 1e-30)
                rs = np_.tile([P, 2], f32, name="rs1")
                nc.vector.reciprocal(rs[:], se[:])
                h_f = np_.tile([P, F_IN], f32, name="h_f")
                hv = h_f[:].rearrange("p (h c) -> p h c", c=HID)
                nc.vector.tensor_tensor(
                    out=hv, in0=ps3[:, :, 0:HID],
                    in1=rs[:].rearrange("p (h o) -> p h o", o=1).to_broadcast(
                        [P, 2, HID]),
                    op=OP.mult)
                nc.vector.tensor_tensor(out=h_f[:], in0=h_f[:], in1=b1rep_s,
                                        op=OP.add)
                rec2_t = np_.tile([P, RECW], bf16, name="rec2t")
                nc.gpsimd.memset(rec2_t[:], 1.0)
                nc.scalar.activation(
                    rec2_t[:].rearrange("p (h q) -> p h q", q=P)[:, :, 4:4 + HID],
                    h_f[:].rearrange("p (h c) -> p h c", c=HID), AF.Relu)
                # relu'd h also needed in f32 for the transpose/alpha matvec
                hr_f = np_.tile([P, F_IN], f32, name="hr_f")
                nc.scalar.activation(hr_f[:], h_f[:], AF.Relu)
                hT_ps = nps.tile([P, P], f32, name="hTps")
                nc.tensor.transpose(out=hT_ps[:], in_=hr_f[:], identity=ident[:])
                hT_s = np_.tile([P, P], bf16, name="hTs")
                nc.vector.tensor_copy(out=hT_s[:], in_=hT_ps[:])
                a2_ps = nps.tile([P, 4], f32, name="a2ps")
                nc.tensor.matmul(out=a2_ps[:], lhsT=hT_s[:], rhs=u2_s[:],
                                 start=True, stop=True)
                nc.vector.tensor_copy(out=rec2_t[:, 0:4], in_=a2_ps[:])
                nc.vector.tensor_copy(out=adtab2[:, T * 2:T * 2 + 2],
                                      in_=a2_ps[:, 2:4])
                nc.sync.dma_start(out=rec2_slice[T * P:(T + 1) * P, :],
                                  in_=rec2_t[:])

            def norm2(ps, T, np_, nps):
                ps3 = ps[:].rearrange("p (g c) -> p g c", c=2 * (HID + 1))
                se = np_.tile([P, 2], f32, name="se2")
                nc.vector.tensor_scalar_add(
                    se[:].rearrange("p (g o) -> p g o", o=1),
                    ps3[:, :, HID:HID + 1], 1e-30)
                rs = np_.tile([P, 2], f32, name="rs2")
                nc.vector.reciprocal(rs[:], se[:])
                agg = np_.tile([P, 2 * F_IN], f32, name="agg")
                nc.vector.tensor_tensor(
                    out=agg[:].rearrange("p (g h c) -> p g h c", g=2, c=HID),
                    in0=ps3[:].rearrange("p g (h c) -> p g h c", c=HID + 1)[
                        :, :, :, 0:HID],
                    in1=rs[:].rearrange("p (g o) -> p g o", o=1)[
                        :, :, :, None].to_broadcast([P, 2, 2, HID]),
                    op=OP.mult)
                rows = min(P, NPC - T * P)
                for gi, (wt_s, brep_s) in enumerate(
                        ((wmut_s, bmurep_s), (wstdt_s, bstdrep_s))):
                    aT_ps = nps.tile([P, P], f32, name="aTps")
                    nc.tensor.transpose(out=aT_ps[:],
                                        in_=agg[:, gi * F_IN:(gi + 1) * F_IN],
                                        identity=ident[:])
                    aT_s = np_.tile([P, P], f32, name="aTs")
                    nc.vector.tensor_copy(out=aT_s[:], in_=aT_ps[:])
                    pr_ps = nps.tile([P, Z], f32, name="prps")
                    nc.tensor.matmul(out=pr_ps[:], lhsT=aT_s[:], rhs=wt_s[:],
                                     start=True, stop=True)
                    o_s = np_.tile([P, Z], f32, name="outs")
                    nc.vector.tensor_tensor(out=o_s[:], in0=pr_ps[:],
                                            in1=brep_s[:], op=OP.add)
                    nc.sync.dma_start(
                        out=musd_out[T * P:T * P + rows,
                                     gi * Z:(gi + 1) * Z],
                        in_=o_s[0:rows, :])

            edge_phase(1, rec1_full, adtab1, norm1)

            # sentinel for layer-2 table (after all norm1 writes)
            nc.sync.dma_start(out=rec2_slice[SENTROW:SENTROW + 1, 0:4],
                              in_=sent_bf[:])

            # ---- AllGather 2 ----
            nc.gpsimd.collective_compute(
                "AllGather", OP.bypass,
                replica_groups=[list(range(NCORES))],
                ins=[rec2_slice[:]], outs=[rec2_full[:]])

            edge_phase(2, rec2_full, adtab2, norm2)

    nc.compile()
    return nc


# ---------------- runner ----------------
_CACHE = {}


def _get_runner(schedule, half_flags):
    key = tuple(schedule.reshape(-1).tolist())
    if key not in _CACHE:
        nc = _build_nc(schedule, half_flags)
        _CACHE[key] = (nc, {})
    return _CACHE[key]


def run_on_hw(inputs_per_core, schedule, half_flags):
    import jax
    from concourse import bass2jax
    nc, captured = _get_runner(schedule, half_flags)
    orig_jit = jax.jit

    def cap_jit(fun, **kw):
        j = orig_jit(fun, **kw)
        captured['fn'] = j
        return j
    jax.jit = cap_jit
    try:
        results = bass2jax.run_bass_via_pjrt(nc, inputs_per_core, n_cores=NCORES)
    finally:
        jax.jit = orig_jit
    return results, captured.get('fn'), nc


def make_inputs_per_core(features, edges, wp):
    schedule, half_flags, per_core = _prep_edges(np.asarray(edges))
    feats = np.asarray(features, np.float32)
    ins = []
    for c in range(NCORES):
        xTs = np.zeros((P, NPCPAD), np.float32)
        xTs[:, 0:NPC] = feats[c * NPC:(c + 1) * NPC].T
        ins.append({"xT": xTs, **wp, **per_core[c]})
    return schedule, half_flags, ins


def kernel(features, edges, W1, a_src1, a_dst1, b1, W_mu, a_src_mu, a_dst_mu,
           b_mu, W_std, a_src_std, a_dst_std, b_std):
    wp = _prep_weights(np.asarray(W1), np.asarray(a_src1), np.asarray(a_dst1),
                       np.asarray(b1), np.asarray(W_mu), np.asarray(a_src_mu),
                       np.asarray(a_dst_mu), np.asarray(b_mu), np.asarray(W_std),
                       np.asarray(a_src_std), np.asarray(a_dst_std),
                       np.asarray(b_std))
    schedule, half_flags, ins = make_inputs_per_core(features, edges, wp)
    results, _, _ = run_on_hw(ins, schedule, half_flags)
    musd = np.concatenate([results[c]["musd_out"] for c in range(NCORES)],
                          axis=0)
    return (np.ascontiguousarray(musd[:, 0:Z]),
            np.ascontiguousarray(musd[:, Z:2 * Z]))


# revision 15
# speedup vs baseline: 1.3909x; 1.0022x over previous
"""GAT (2-layer, mu/std heads) Trainium2 kernel — 8-core SPMD.

Sharding: nodes partitioned into 8 contiguous ranges (dst-sharding); edges
assigned to the core owning their dst, sorted by (dst-tile, src-half, src).
Per-layer halo exchange of bf16 node records via AllGather. Edge gather via
dma_gather (512B records by src from the global table). alpha_dst is fetched
on-chip: a transposed one-hot (dst-slot x edge) matmuls a per-tile alpha_dst
table, so no second DMA gather is needed. Scatter-add via one-hot matmul
with softmax denominators as extra matmul columns; W_mu/W_std projections
applied after aggregation.

Record layout (bf16, 256 elems = 512B):
  [0]=as0 [1]=as1 [2]=ad0 [3]=ad1 [4:68]=x_h0 [68]=1.0
  [132:196]=x_h1 [196]=1.0  (rest pad; layer2: h0/h1 are halves of h)
"""
import sys
sys.path.insert(0, '/opt/trn_rl_repo')
import numpy as np
import ml_dtypes

BF = ml_dtypes.bfloat16

# ---------------- problem constants (hardcoded per spec) ----------------
N = 50000
F_IN = 128
HID = 64
H = 2
Z = 32
NEG = 0.2
NCORES = 8
NPC = N // NCORES            # 6250 nodes per core
P = 128
NT = (NPC + P - 1) // P      # 49 dst tiles per core
NPCPAD = NT * P              # 6272
SENTROW = NPCPAD - 1         # per-core sentinel row (alpha = -1e30)
RECW = 256                   # record bf16 elems per node row (512 B)
HALFROWS = (NCORES // 2) * NPCPAD   # 25088 rows per half-table
NB = 32                      # blocks per gather batch
GMAXB = 8                    # max blocks (128 idx each) per dma_gather call
BIG = -1.0e30


# ---------------- host-side prep ----------------
def _prep_edges(edges):
    """Shard + sort by (tile, src-half, src) + pad; build packed index arrays.

    Returns (schedule [NT,2] int, half_flags, per-core dict)."""
    src = np.concatenate([edges[0].astype(np.int64), np.arange(N, dtype=np.int64)])
    dst = np.concatenate([edges[1].astype(np.int64), np.arange(N, dtype=np.int64)])
    core = dst // NPC
    dstl = dst - core * NPC
    tile = dstl >> 7
    src_row = (src // NPC) * NPCPAD + (src % NPC)   # padded global row
    half = (src_row >= HALFROWS).astype(np.int64)

    counts = np.zeros((NCORES, NT, 2), np.int64)
    np.add.at(counts, (core, tile, half), 1)
    blocks = (counts + P - 1) // P                   # [C, NT, 2]
    schedule = blocks.max(axis=0)                    # [NT, 2]
    schedule[:, 0] = np.maximum(schedule[:, 0], 1)   # >=1 block per tile
    nblk = int(schedule.sum())

    # flat block index of each (tile, half) group start
    grp_blocks = schedule.reshape(-1)                # [NT*2]
    grp_start = np.zeros(NT * 2, np.int64)
    grp_start[1:] = np.cumsum(grp_blocks)[:-1]
    grp_start = grp_start.reshape(NT, 2)

    half_flags = np.zeros(nblk, np.int64)
    for t in range(NT):
        half_flags[grp_start[t, 1]:grp_start[t, 1] + schedule[t, 1]] = 1

    per_core = []
    for c in range(NCORES):
        m = core == c
        key = tile[m] * 2 + half[m]
        srow_m = src_row[m]
        order = np.lexsort((srow_m, key))            # by group, then src asc
        key_s = key[order]
        srow_s = srow_m[order]
        dstl_s = dstl[m][order]
        half_s = half[m][order]
        cnt = counts[c].reshape(-1)                  # [NT*2]
        g0 = np.zeros(NT * 2, np.int64)
        g0[1:] = np.cumsum(cnt)[:-1]
        k = np.arange(key_s.size, dtype=np.int64) - g0[key_s]
        flat = (grp_start.reshape(-1)[key_s] + (k >> 7)) * P + (k & 127)

        # defaults: pad edges -> sentinel record of the matching half
        esrc = np.empty(nblk * P, np.int64)
        for t in range(NT):
            a0, a1 = grp_start[t, 0] * P, (grp_start[t, 0] + schedule[t, 0]) * P
            b0, b1 = grp_start[t, 1] * P, (grp_start[t, 1] + schedule[t, 1]) * P
            esrc[a0:a1] = SENTROW                    # core0 sentinel (half A)
            esrc[b0:b1] = (NCORES // 2) * NPCPAD + SENTROW - HALFROWS
        eslot = np.zeros(nblk * P, np.int64)
        esrc[flat] = srow_s - half_s * HALFROWS
        eslot[flat] = dstl_s & 127

        # gather-idx wrap-16 packing, replicated to 128 partitions
        def pack16(vals):
            n = vals.size
            t16 = np.zeros((16, n // 16), np.int16)
            t16[np.arange(n) % 16, np.arange(n) // 16] = vals.astype(np.int16)
            return np.ascontiguousarray(np.tile(t16, (8, 1)))

        per_core.append({
            "esrc16": pack16(esrc),
            "eslot": np.ascontiguousarray(
                eslot.reshape(nblk, P).T.astype(BF)),
            "eslotf": np.ascontiguousarray(eslot.astype(BF).reshape(1, -1)),
        })
    return schedule, half_flags, per_core


def _prep_weights(W1, a_src1, a_dst1, b1, W_mu, a_src_mu, a_dst_mu, b_mu,
                  W_std, a_src_std, a_dst_std, b_std):
    am1 = np.zeros((F_IN, 4), np.float32)
    am1[0:HID, 0] = a_src1[0]
    am1[HID:2 * HID, 1] = a_src1[1]
    am1[0:HID, 2] = a_dst1[0]
    am1[HID:2 * HID, 3] = a_dst1[1]
    am2 = np.zeros((2 * Z, 4), np.float32)
    am2[0:Z, 0] = a_src_mu[0]
    am2[Z:2 * Z, 1] = a_src_std[0]
    am2[0:Z, 2] = a_dst_mu[0]
    am2[Z:2 * Z, 3] = a_dst_std[0]
    # pack all node-local params into one [128, 648] f32 blob:
    # cols: w1t 0:128 | w1raw 128:256 | amask1 256:260 | wcat 260:388 (rows<64)
    #       amask2 388:392 (rows<64) | wmut 392:424 | wstdt 424:456
    #       b1rep 456:584 | bmurep 584:616 | bstdrep 616:648
    blob = np.zeros((P, 648), np.float32)
    blob[:, 0:128] = W1.T
    blob[:, 128:256] = W1
    blob[:, 256:260] = am1
    blob[0:2 * Z, 260:388] = np.vstack([W_mu, W_std])
    blob[0:2 * Z, 388:392] = am2
    blob[:, 392:424] = W_mu.T
    blob[:, 424:456] = W_std.T
    blob[:, 456:584] = np.tile(b1.astype(np.float32), (P, 1))
    blob[:, 584:616] = np.tile(b_mu.astype(np.float32), (P, 1))
    blob[:, 616:648] = np.tile(b_std.astype(np.float32), (P, 1))
    return {"wblob": np.ascontiguousarray(blob)}


# ---------------- device program ----------------
def _build_nc(schedule, half_flags):
    import concourse.bass as bass
    import concourse.mybir as mybir
    import concourse.tile as tile
    import concourse.bacc as bacc
    from concourse.masks import make_identity

    f32 = mybir.dt.float32
    bf16 = mybir.dt.bfloat16
    i16 = mybir.dt.int16
    AF = mybir.ActivationFunctionType
    OP = mybir.AluOpType
    nblk = int(schedule.sum())

    # flat block index -> dst tile
    tile_of_block = np.zeros(nblk, np.int64)
    b = 0
    for t in range(NT):
        for hf in range(2):
            for _ in range(int(schedule[t, hf])):
                tile_of_block[b] = t
                b += 1

    nc = bacc.Bacc("TRN2", target_bir_lowering=False, debug=False,
                   num_devices=NCORES)

    # ---- I/O ----
    xT = nc.dram_tensor("xT", [P, NPCPAD], f32, kind="ExternalInput")
    wblob_d = nc.dram_tensor("wblob", [P, 648], f32, kind="ExternalInput")
    esrc_d = nc.dram_tensor("esrc16", [P, nblk * 8], i16, kind="ExternalInput")
    eslot_d = nc.dram_tensor("eslot", [P, nblk], bf16, kind="ExternalInput")
    eslotf_d = nc.dram_tensor("eslotf", [1, nblk * P], bf16, kind="ExternalInput")
    musd_out = nc.dram_tensor("musd_out", [NPC, 2 * Z], f32, kind="ExternalOutput")

    with tile.TileContext(nc) as tc:
        with tc.tile_pool(name="dram", bufs=1, space="DRAM") as dram, \
             tc.tile_pool(name="const", bufs=1) as cp:
            rec1_slice = dram.tile([NPCPAD, RECW], bf16)
            rec1_full = dram.tile([NPCPAD * NCORES, RECW], bf16,
                                  addr_space="Shared")
            rec2_slice = dram.tile([NPCPAD, RECW], bf16)
            rec2_full = dram.tile([NPCPAD * NCORES, RECW], bf16,
                                  addr_space="Shared")

            # ---- constants ----
            iota_bf = cp.tile([P, P], bf16)
            nc.gpsimd.iota(iota_bf[:], pattern=[[1, P]], base=0,
                           channel_multiplier=0,
                           allow_small_or_imprecise_dtypes=True)
            iota_col = cp.tile([P, 1], f32)
            nc.gpsimd.iota(iota_col[:], pattern=[[1, 1]], base=0,
                           channel_multiplier=1,
                           allow_small_or_imprecise_dtypes=True)
            ident = cp.tile([P, P], f32)
            make_identity(nc, ident[:])
            sent_bf = cp.tile([1, 4], bf16)
            nc.gpsimd.memset(sent_bf[:], BIG)
            # per-tile alpha_dst tables (bf16), filled by the node phases
            adtab1 = cp.tile([P, NT * 2], bf16)
            adtab2 = cp.tile([P, NT * 2], bf16)

            wblob_s = cp.tile([P, 648], f32, name="wblob_s")
            nc.sync.dma_start(out=wblob_s[:], in_=wblob_d[:])
            w1t_s = wblob_s[:, 0:128]
            w1raw_s = wblob_s[:, 128:256]
            amask1_s = wblob_s[:, 256:260]
            wcat_s = wblob_s[0:2 * Z, 260:388]
            amask2_s = wblob_s[0:2 * Z, 388:392]
            wmut_s = wblob_s[:, 392:424]
            wstdt_s = wblob_s[:, 424:456]
            b1rep_s = wblob_s[:, 456:584]
            bmurep_s = wblob_s[:, 584:616]
            bstdrep_s = wblob_s[:, 616:648]

            # ---- u-vectors (alpha matvec weights), bf16 copies for L2 ----
            u1_s = cp.tile([F_IN, 4], f32)
            u2_s = cp.tile([F_IN, 4], bf16)
            with tc.tile_pool(name="ups", bufs=1, space="PSUM") as ups:
                u1_ps = ups.tile([F_IN, 4], f32)
                nc.tensor.matmul(out=u1_ps[:], lhsT=w1raw_s, rhs=amask1_s,
                                 start=True, stop=True)
                nc.vector.tensor_copy(out=u1_s[:], in_=u1_ps[:])
                u2_ps = ups.tile([F_IN, 4], f32)
                nc.tensor.matmul(out=u2_ps[:], lhsT=wcat_s,
                                 rhs=amask2_s, start=True, stop=True)
                nc.vector.tensor_copy(out=u2_s[:], in_=u2_ps[:])

            # ---- node phase 1 ----
            with tc.tile_pool(name="xtp", bufs=1) as xtp, \
                 tc.tile_pool(name="n1", bufs=3) as n1, \
                 tc.tile_pool(name="n1ps", bufs=2, space="PSUM") as n1ps:
                xT_s = xtp.tile([P, NPCPAD], f32)
                nc.sync.dma_start(out=xT_s[:], in_=xT[:])
                for T in range(NT):
                    lhs = xT_s[:, T * P:(T + 1) * P]
                    xp_ps = n1ps.tile([P, F_IN], f32)
                    a1_ps = n1ps.tile([P, 4], f32)
                    nc.tensor.matmul(out=xp_ps[:], lhsT=lhs, rhs=w1t_s,
                                     start=True, stop=True)
                    nc.tensor.matmul(out=a1_ps[:], lhsT=lhs, rhs=u1_s[:],
                                     start=True, stop=True)
                    rec_t = n1.tile([P, RECW], bf16)
                    nc.gpsimd.memset(rec_t[:], 1.0)
                    nc.vector.tensor_copy(
                        out=rec_t[:].rearrange("p (h q) -> p h q", q=P)[:, :, 4:4 + HID],
                        in_=xp_ps[:].rearrange("p (h c) -> p h c", c=HID))
                    nc.vector.tensor_copy(out=rec_t[:, 0:4], in_=a1_ps[:])
                    nc.vector.tensor_copy(out=adtab1[:, T * 2:T * 2 + 2],
                                          in_=a1_ps[:, 2:4])
                    nc.sync.dma_start(out=rec1_slice[T * P:(T + 1) * P, :],
                                      in_=rec_t[:])
                nc.sync.dma_start(out=rec1_slice[SENTROW:SENTROW + 1, 0:4],
                                  in_=sent_bf[:])

            # ---- AllGather 1 ----
            nc.gpsimd.collective_compute(
                "AllGather", OP.bypass,
                replica_groups=[list(range(NCORES))],
                ins=[rec1_slice[:]], outs=[rec1_full[:]])

            # ---- edge phase (shared for both layers) ----
            def edge_phase(layer, full_tab, adtab, normalize):
                # w width per block: L1 2*(65) ; L2 2*2*(65)
                ngrp = 2 if layer == 2 else 1
                ww = ngrp * 2 * (HID + 1)            # 130 / 260
                viewA = full_tab[0:HALFROWS, :]
                viewB = full_tab[HALFROWS:2 * HALFROWS, :]
                with tc.tile_pool(name=f"e{layer}", bufs=3) as ep, \
                     tc.tile_pool(name=f"e{layer}a", bufs=2) as epa, \
                     tc.tile_pool(name=f"n{layer}x", bufs=3) as np_, \
                     tc.tile_pool(name=f"e{layer}ps", bufs=2, space="PSUM") as eps, \
                     tc.tile_pool(name=f"ad{layer}ps", bufs=2, space="PSUM") as adps_p, \
                     tc.tile_pool(name=f"n{layer}xps", bufs=2, space="PSUM") as nps:
                    state = {"a0": None, "w": None, "b0": 0}

                    def emit_batch(b0):
                        bn = min(NB, nblk - b0)
                        esrc_t = ep.tile([P, NB * 8], i16, name=f"esrc{layer}")
                        dslot = ep.tile([P, NB], bf16, name=f"dslot{layer}")
                        eslotT = ep.tile([P, NB * P], bf16, name=f"eslT{layer}")
                        nc.sync.dma_start(out=esrc_t[:, 0:bn * 8],
                                          in_=esrc_d[:, b0 * 8:(b0 + bn) * 8])
                        nc.sync.dma_start(out=dslot[:, 0:bn],
                                          in_=eslot_d[:, b0:b0 + bn])
                        nc.sync.dma_start(
                            out=eslotT[:, 0:bn * P],
                            in_=eslotf_d[0:1, b0 * P:(b0 + bn) * P].to_broadcast(
                                [P, bn * P]))
                        rec_g = ep.tile([P, NB * RECW], bf16, name=f"rec_g{layer}")
                        # gather: per same-half run of blocks (alternate queues)
                        r0 = 0
                        while r0 < bn:
                            hf = half_flags[b0 + r0]
                            r1 = r0 + 1
                            while (r1 < bn and r1 - r0 < GMAXB
                                   and half_flags[b0 + r1] == hf):
                                r1 += 1
                            nrun = (r1 - r0) * P
                            nc.gpsimd.dma_gather(
                                out_ap=rec_g[:, r0 * RECW:r1 * RECW].rearrange(
                                    "p (g e) -> p g e", e=RECW),
                                in_ap=(viewB if hf else viewA),
                                idxs_ap=esrc_t[:, r0 * 8:r1 * 8],
                                num_idxs=nrun, num_idxs_reg=nrun,
                                elem_size=RECW)
                            r0 = r1
                        # transposed one-hot (dst-slot x edge): per-partition
                        # scalar compare -> DVE 4x fast path
                        a0T = epa.tile([P, NB * P], bf16, name=f"a0T{layer}")
                        nc.vector.tensor_scalar(
                            out=a0T[:, 0:bn * P],
                            in0=eslotT[:, 0:bn * P],
                            scalar1=iota_col[:, 0:1], scalar2=None,
                            op0=OP.is_equal)
                        ad_ps = adps_p.tile([P, NB * 2], f32, name=f"adps{layer}")
                        for o in range(bn):
                            t = int(tile_of_block[b0 + o])
                            nc.tensor.matmul(
                                out=ad_ps[:, o * 2:(o + 1) * 2],
                                lhsT=a0T[:, o * P:(o + 1) * P],
                                rhs=adtab[:, t * 2:t * 2 + 2],
                                start=True, stop=True)
                        # t = as + ad ; u = max(.2t, t) ; p = exp(u)
                        tt = ep.tile([P, NB * 2], bf16, name=f"tt{layer}")
                        nc.vector.tensor_tensor(
                            out=tt[:, 0:bn * 2].rearrange("p (b h) -> p b h", h=2),
                            in0=rec_g[:, 0:bn * RECW].rearrange(
                                "p (b r) -> p b r", r=RECW)[:, :, 0:2],
                            in1=ad_ps[:, 0:bn * 2].rearrange(
                                "p (b h) -> p b h", h=2),
                            op=OP.add)
                        uu = ep.tile([P, NB * 2], bf16, name=f"uu{layer}")
                        nc.vector.scalar_tensor_tensor(
                            out=uu[:, 0:bn * 2], in0=tt[:, 0:bn * 2],
                            scalar=NEG, in1=tt[:, 0:bn * 2],
                            op0=OP.mult, op1=OP.max)
                        pp = ep.tile([P, NB * 2], bf16, name=f"pp{layer}")
                        nc.scalar.activation(pp[:, 0:bn * 2], uu[:, 0:bn * 2],
                                             AF.Exp)
                        # A0 one-hot
                        a0 = epa.tile([P, NB * P], bf16, name=f"a0_{layer}")
                        nc.vector.tensor_tensor(
                            out=a0[:, 0:bn * P].rearrange("p (b r) -> p b r", r=P),
                            in0=dslot[:, 0:bn][:, :, None].to_broadcast([P, bn, P]),
                            in1=iota_bf[:][:, None, :].to_broadcast([P, bn, P]),
                            op=OP.is_equal)
                        # w build
                        w = epa.tile([P, NB * ww], bf16, name=f"w{layer}")
                        rec3 = rec_g[:, 0:bn * RECW].rearrange(
                            "p (b r) -> p b r", r=RECW)
                        rec4 = rec3.rearrange("p b (h q) -> p b h q", q=P)[
                            :, :, :, 4:4 + HID + 1]
                        if layer == 1:
                            in1 = pp[:, 0:bn * 2].rearrange(
                                "p (b h) -> p b h", h=2)[:, :, :, None].to_broadcast(
                                [P, bn, 2, HID + 1])
                            wv = w[:, 0:bn * ww].rearrange(
                                "p (b h c) -> p b h c", h=2, c=HID + 1)
                            nc.vector.tensor_tensor(out=wv, in0=rec4, in1=in1,
                                                    op=OP.mult)
                        else:
                            pp3 = pp[:, 0:bn * 2].rearrange(
                                "p (b g) -> p b g", g=2)
                            wv4 = w[:, 0:bn * ww].rearrange(
                                "p (b g hc) -> p b g hc", g=2, hc=2 * (HID + 1))
                            for g, eng in ((0, nc.vector), (1, nc.vector)):
                                eng.tensor_tensor(
                                    out=wv4[:, :, g].rearrange(
                                        "p b (h c) -> p b h c", c=HID + 1),
                                    in0=rec4,
                                    in1=pp3[:, :, g:g + 1][:, :, :, None].to_broadcast(
                                        [P, bn, 2, HID + 1]),
                                    op=OP.mult)
                        state["a0"], state["w"], state["b0"] = a0, w, b0

                    B = 0
                    for T in range(NT):
                        ps = eps.tile([P, ww], f32, name=f"acc{layer}")
                        kb = int(schedule[T].sum())
                        for j in range(kb):
                            if state["a0"] is None or B >= state["b0"] + NB:
                                emit_batch(B)
                            o = B - state["b0"]
                            nc.tensor.matmul(
                                out=ps[:],
                                lhsT=state["a0"][:, o * P:(o + 1) * P],
                                rhs=state["w"][:, o * ww:(o + 1) * ww],
                                start=(j == 0), stop=(j == kb - 1))
                            B += 1
                        normalize(ps, T, np_, nps)

            # ---- normalize callbacks ----
            def norm1(ps, T, np_, nps):
                ps3 = ps[:].rearrange("p (h c) -> p h c", c=HID + 1)
                se = np_.tile([P, 2], f32, name="se1")
                nc.vector.tensor_scalar_add(
                    se[:].rearrange("p (h o) -> p h o", o=1),
                    ps3[:, :, HID:HID + 1], 1e-30)
                rs = np_.tile([P, 2], f32, name="rs1")
                nc.vector.reciprocal(rs[:], se[:])
                h_f = np_.tile([P, F_IN], f32, name="h_f")
                hv = h_f[:].rearrange("p (h c) -> p h c", c=HID)
                nc.vector.tensor_tensor(
                    out=hv, in0=ps3[:, :, 0:HID],
                    in1=rs[:].rearrange("p (h o) -> p h o", o=1).to_broadcast(
                        [P, 2, HID]),
                    op=OP.mult)
                nc.vector.tensor_tensor(out=h_f[:], in0=h_f[:], in1=b1rep_s,
                                        op=OP.add)
                rec2_t = np_.tile([P, RECW], bf16, name="rec2t")
                nc.gpsimd.memset(rec2_t[:], 1.0)
                nc.scalar.activation(
                    rec2_t[:].rearrange("p (h q) -> p h q", q=P)[:, :, 4:4 + HID],
                    h_f[:].rearrange("p (h c) -> p h c", c=HID), AF.Relu)
                # relu'd h also needed in f32 for the transpose/alpha matvec
                hr_f = np_.tile([P, F_IN], f32, name="hr_f")
                nc.scalar.activation(hr_f[:], h_f[:], AF.Relu)
                hT_ps = nps.tile([P, P], f32, name="hTps")
                nc.tensor.transpose(out=hT_ps[:], in_=hr_f[:], identity=ident[:])
                hT_s = np_.tile([P, P], bf16, name="hTs")
                nc.vector.tensor_copy(out=hT_s[:], in_=hT_ps[:])
                a2_ps = nps.tile([P, 4], f32, name="a2ps")
                nc.tensor.matmul(out=a2_ps[:], lhsT=hT_s[:], rhs=u2_s[:],
                                 start=True, stop=True)
                nc.vector.tensor_copy(out=rec2_t[:, 0:4], in_=a2_ps[:])
                nc.vector.tensor_copy(out=adtab2[:, T * 2:T * 2 + 2],
                                      in_=a2_ps[:, 2:4])
                nc.sync.dma_start(out=rec2_slice[T * P:(T + 1) * P, :],
                                  in_=rec2_t[:])

            def norm2(ps, T, np_, nps):
                ps3 = ps[:].rearrange("p (g c) -> p g c", c=2 * (HID + 1))
                se = np_.tile([P, 2], f32, name="se2")
                nc.vector.tensor_scalar_add(
                    se[:].rearrange("p (g o) -> p g o", o=1),
                    ps3[:, :, HID:HID + 1], 1e-30)
                rs = np_.tile([P, 2], f32, name="rs2")
                nc.vector.reciprocal(rs[:], se[:])
                agg = np_.tile([P, 2 * F_IN], f32, name="agg")
                nc.vector.tensor_tensor(
                    out=agg[:].rearrange("p (g h c) -> p g h c", g=2, c=HID),
                    in0=ps3[:].rearrange("p g (h c) -> p g h c", c=HID + 1)[
                        :, :, :, 0:HID],
                    in1=rs[:].rearrange("p (g o) -> p g o", o=1)[
                        :, :, :, None].to_broadcast([P, 2, 2, HID]),
                    op=OP.mult)
                rows = min(P, NPC - T * P)
                for gi, (wt_s, brep_s) in enumerate(
                        ((wmut_s, bmurep_s), (wstdt_s, bstdrep_s))):
                    aT_ps = nps.tile([P, P], f32, name="aTps")
                    nc.tensor.transpose(out=aT_ps[:],
                                        in_=agg[:, gi * F_IN:(gi + 1) * F_IN],
                                        identity=ident[:])
                    aT_s = np_.tile([P, P], f32, name="aTs")
                    nc.vector.tensor_copy(out=aT_s[:], in_=aT_ps[:])
                    pr_ps = nps.tile([P, Z], f32, name="prps")
                    nc.tensor.matmul(out=pr_ps[:], lhsT=aT_s[:], rhs=wt_s[:],
                                     start=True, stop=True)
                    o_s = np_.tile([P, Z], f32, name="outs")
                    nc.vector.tensor_tensor(out=o_s[:], in0=pr_ps[:],
                                            in1=brep_s[:], op=OP.add)
                    nc.sync.dma_start(
                        out=musd_out[T * P:T * P + rows,
                                     gi * Z:(gi + 1) * Z],
                        in_=o_s[0:rows, :])

            edge_phase(1, rec1_full, adtab1, norm1)

            # sentinel for layer-2 table (after all norm1 writes)
            nc.sync.dma_start(out=rec2_slice[SENTROW:SENTROW + 1, 0:4],
                              in_=sent_bf[:])

            # ---- AllGather 2 ----
            nc.gpsimd.collective_compute(
                "AllGather", OP.bypass,
                replica_groups=[list(range(NCORES))],
                ins=[rec2_slice[:]], outs=[rec2_full[:]])

            edge_phase(2, rec2_full, adtab2, norm2)

    nc.compile()
    return nc


# ---------------- runner ----------------
_CACHE = {}


def _get_runner(schedule, half_flags):
    key = tuple(schedule.reshape(-1).tolist())
    if key not in _CACHE:
        nc = _build_nc(schedule, half_flags)
        _CACHE[key] = (nc, {})
    return _CACHE[key]


def run_on_hw(inputs_per_core, schedule, half_flags):
    import jax
    from concourse import bass2jax
    nc, captured = _get_runner(schedule, half_flags)
    orig_jit = jax.jit

    def cap_jit(fun, **kw):
        j = orig_jit(fun, **kw)
        captured['fn'] = j
        return j
    jax.jit = cap_jit
    try:
        results = bass2jax.run_bass_via_pjrt(nc, inputs_per_core, n_cores=NCORES)
    finally:
        jax.jit = orig_jit
    return results, captured.get('fn'), nc


def make_inputs_per_core(features, edges, wp):
    schedule, half_flags, per_core = _prep_edges(np.asarray(edges))
    feats = np.asarray(features, np.float32)
    ins = []
    for c in range(NCORES):
        xTs = np.zeros((P, NPCPAD), np.float32)
        xTs[:, 0:NPC] = feats[c * NPC:(c + 1) * NPC].T
        ins.append({"xT": xTs, **wp, **per_core[c]})
    return schedule, half_flags, ins


def kernel(features, edges, W1, a_src1, a_dst1, b1, W_mu, a_src_mu, a_dst_mu,
           b_mu, W_std, a_src_std, a_dst_std, b_std):
    wp = _prep_weights(np.asarray(W1), np.asarray(a_src1), np.asarray(a_dst1),
                       np.asarray(b1), np.asarray(W_mu), np.asarray(a_src_mu),
                       np.asarray(a_dst_mu), np.asarray(b_mu), np.asarray(W_std),
                       np.asarray(a_src_std), np.asarray(a_dst_std),
                       np.asarray(b_std))
    schedule, half_flags, ins = make_inputs_per_core(features, edges, wp)
    results, _, _ = run_on_hw(ins, schedule, half_flags)
    musd = np.concatenate([results[c]["musd_out"] for c in range(NCORES)],
                          axis=0)
    return (np.ascontiguousarray(musd[:, 0:Z]),
            np.ascontiguousarray(musd[:, Z:2 * Z]))
